# revision 7
# baseline (speedup 1.0000x reference)
"""Cached multi-head attention decode step — raw-bacc Trainium2 kernel.

Math: the KV/Q caches are all-zero except slot 0, so the S x S attention
collapses exactly:
  out[b, 0,   h*D+d] = w_bh * v[b,h,d],   w_bh = sigmoid(q.k*SCALE - ln(S-1))
  out[b, s>0, h*D+d] = v[b,h,d] / S
(softmax of an all-zero row is uniform 1/S; only cache row 0 of V is nonzero.)

Sharding: 8 cores = 4 head-groups (3 heads, 192 output cols) x 2 batch-pairs.
Host assembly: bulk tensor covers rows 1..2047 (its row 0 is a don't-care);
a small row0 tensor is overlaid on top.

Raw bacc (no TileContext) with hand-wired semaphores:
  - Bulk rows ride one batch=2 kv_writeback whose descriptors are
    pre-generated on the idle Pool engine (SWDGE prepare-only); a trigger
    fires them the moment the bf16 source tile (both batches' v-row on all
    128 partitions, dho-broadcast in-AP) is ready. No HWDGE hold or DGE
    delay sits between data-ready and bytes-moving.
  - Row 0 = sigmoid(q.k*SCALE - ln(S-1)) times an unscaled v projection,
    PE-transposed into a partition-pair layout and written by a third
    prepared kv_writeback (d_head=256, ncn=1) into its own small tensor —
    no HWDGE latency on this tail either. (dma_scatter_add is rejected by
    this environment's runtime.)
  - V path and outputs are bf16; Wq/Wk/xq/xk are fp8(e4m3) with host-side
    x64/x16 prescales folded into the sigmoid input scale. End-to-end rel
    err vs the f32 reference: 4.5e-3 (tolerance 2e-2).
"""

import math
import threading

import numpy as np

B, H, S, D, E = 4, 12, 2048, 64, 768
SCALE = D**-0.5
P = 128
NCH = E // P  # 6
HG = 3
M = HG * D  # 192
N_CORES = 8

W8_SCALE = 64.0
X8_SCALE = 16.0
SIG_SCALE = SCALE / (W8_SCALE * W8_SCALE * X8_SCALE * X8_SCALE)
SIG_BIAS = -math.log(S - 1)

PA_COLS = 4 * NCH + NCH * M  # bf16: [xv | wv | xv2(unscaled)]
XV2 = 2 * NCH + NCH * M
# fp8 tensor: [ctxi(8B) | sidx(16B) | xq | xk | wq | wk]
CTXI0, SIDX0 = 0, 8
XQ0 = 24
XK0 = XQ0 + 2 * NCH
WQ0 = XK0 + 2 * NCH
WK0 = WQ0 + NCH * M
PC_COLS = WK0 + NCH * M

PA_SPLIT = 4  # wv chunks in the first pa piece
WARMUP_MMS = 6
WARM_COLS = 512

_lock = threading.Lock()
_nc_cache = {}
LAST_RESULTS = None


def _build_nc():
    import concourse.mybir as mybir
    from concourse import bacc

    f32 = mybir.dt.float32
    bf16 = mybir.dt.bfloat16
    fp8 = mybir.dt.float8e4
    i32 = mybir.dt.int32
    i16 = mybir.dt.int16
    AX = mybir.AxisListType
    ALU = mybir.AluOpType
    ACTF = mybir.ActivationFunctionType

    nc = bacc.Bacc("TRN2", target_bir_lowering=False, debug=False)
    pa_d = nc.declare_dram_parameter("pa", [P, PA_COLS], bf16, isOutput=False)
    pc_d = nc.declare_dram_parameter("pc", [P, PC_COLS], fp8, isOutput=False)
    out_d = nc.declare_dram_parameter("out", [2, S, M], bf16, isOutput=True)
    row0_d = nc.declare_dram_parameter("row0", [2, 256], bf16, isOutput=True)

    # SBUF
    pa_sb = nc.alloc_sbuf_tensor("pa_sb", [P, PA_COLS], bf16)
    pc_sb = nc.alloc_sbuf_tensor("pc_sb", [P, PC_COLS], fp8)
    wu = nc.alloc_sbuf_tensor("wu", [P, WARM_COLS], bf16)  # garbage, never init
    sel = nc.alloc_sbuf_tensor("sel", [2, 2 * P + 2], bf16)
    sel_d = nc.declare_dram_parameter("seld", [2, 2 * P + 2], bf16, isOutput=False)
    bias_sb = nc.alloc_sbuf_tensor("bias_sb", [2, 1], f32)
    dum = nc.alloc_sbuf_tensor("dum", [2, 1], f32)  # dummy act in/out
    vrow_sb = nc.alloc_sbuf_tensor("vrow_sb", [2, M], bf16)
    vb = nc.alloc_sbuf_tensor("vb", [P, 2 * M], bf16)
    row0_sb = nc.alloc_sbuf_tensor("row0_sb", [P, 256], bf16)
    qk_sb = nc.alloc_sbuf_tensor("qk_sb", [2, M], f32)
    q_sb = nc.alloc_sbuf_tensor("q_sb", [2, M], f32)
    s3 = nc.alloc_sbuf_tensor("s3", [2, HG], f32)
    w2 = nc.alloc_sbuf_tensor("w2", [2, HG], f32)
    w2s = nc.alloc_sbuf_tensor("w2s", [2, HG], f32)

    # PSUM
    wu_ps = nc.alloc_psum_tensor("wu_ps", [P, WARM_COLS], f32)
    q_ps = nc.alloc_psum_tensor("q_ps", [2, M], f32)
    k_ps = nc.alloc_psum_tensor("k_ps", [2, M], f32)
    v_ps = nc.alloc_psum_tensor("v_ps", [2, M], f32)
    v2_ps = nc.alloc_psum_tensor("v2_ps", [2, M], f32)
    vbr = nc.alloc_sbuf_tensor("vbr", [P, 4], bf16)
    pbA = nc.alloc_psum_tensor("pbA", [P, M], f32)
    pbB = nc.alloc_psum_tensor("pbB", [P, M], f32)

    # semaphores
    sem = nc.alloc_semaphore
    s_pc1, s_pc2 = sem("s_pc1"), sem("s_pc2")
    s_pa1, s_pa2 = sem("s_pa1"), sem("s_pa2")
    s_r0d = sem("s_r0d")
    s_v2 = sem("s_v2")
    s_t1, s_t2 = sem("s_t1"), sem("s_t2")
    s_vbr = sem("s_vbr")
    pq3 = sem("pq3")
    r0_sem = sem("r0_dma")
    s_sel = sem("s_sel")
    s_q = sem("s_q")
    s_qsb = sem("s_qsb")
    s_qk = sem("s_qk")
    s_vps = sem("s_vps")
    s_vrow = sem("s_vrow")
    s_pbA, s_pbB = sem("s_pbA"), sem("s_pbB")
    s_vb0, s_vb1 = sem("s_vb0"), sem("s_vb1")
    s_s3 = sem("s_s3")
    s_w2s = sem("s_w2s")
    s_row0 = sem("s_row0")
    pq1, pq2 = sem("pq1"), sem("pq2")
    kv_sem, sc_sem = sem("kv_dma"), sem("sc_dma")

    # ---- SP: input DMAs (pc split for early q/k start, then pa split) ----
    cut1 = WK0  # piece 1 = idx words + xq + xk + wq
    nc.sync.dma_start(pc_sb[:, 0:cut1], pc_d[:, 0:cut1]).then_inc(s_pc1, 16)
    nc.sync.dma_start(pc_sb[:, cut1:PC_COLS], pc_d[:, cut1:PC_COLS]).then_inc(
        s_pc2, 16
    )
    pa_cut = 2 * NCH + PA_SPLIT * M
    nc.sync.dma_start(pa_sb[:, 0:pa_cut], pa_d[:, 0:pa_cut]).then_inc(s_pa1, 16)
    nc.sync.dma_start(pa_sb[:, pa_cut:PA_COLS], pa_d[:, pa_cut:PA_COLS]).then_inc(
        s_pa2, 16
    )
    # sel, last on the SP ring (off the critical path)
    nc.sync.dma_start(sel[:, :], sel_d[:, :]).then_inc(s_sel, 16)

    # ---- DVE: constants, then dots, then v-row copies ----
    nc.vector.memset(bias_sb[:, :], SIG_BIAS)
    nc.vector.wait_ge(s_qsb, 1)
    nc.vector.wait_ge(s_qk, 1)
    nc.vector.tensor_mul(qk_sb[:, :], q_sb[:, :], q_ps[:, :])
    nc.vector.tensor_reduce(
        s3[:, :],
        qk_sb[:, :].rearrange("p (h d) -> p h d", d=D),
        axis=AX.X,
        op=ALU.add,
    ).then_inc(s_s3, 1)
    nc.vector.wait_ge(s_vps, 1)
    nc.vector.tensor_copy(vrow_sb[:, :], v_ps[:, :]).then_inc(s_vrow, 1)
    nc.vector.wait_ge(s_w2s, 1)
    nc.vector.wait_ge(s_v2, 1)
    nc.vector.tensor_tensor(
        row0_sb[0:2, 0:M],
        v2_ps[:, :].rearrange("p (h d) -> p h d", d=D),
        w2[:, :].rearrange("p (h d) -> p h d", d=1).broadcast_to([2, HG, D]),
        op=ALU.mult,
    ).then_inc(s_row0, 1)
    nc.vector.wait_ge(s_pbB, 1)
    nc.vector.tensor_copy(vb[:, M : 2 * M], pbB[:, :]).then_inc(s_vb1, 1)
    nc.vector.wait_ge(s_t1, 1)
    nc.vector.tensor_copy(vbr[0:96, 0:2], wu_ps[0:96, 0:2])
    nc.vector.wait_ge(s_t2, 1)
    nc.vector.tensor_copy(vbr[0:96, 2:4], wu_ps[0:96, 2:4]).then_inc(s_vbr, 1)

    # ---- ACT: act-table warm (Sigmoid set), zero pad, sigmoid, w2s, vb b1 ----
    nc.scalar.activation(dum[:, :], bias_sb[:, :], ACTF.Sigmoid)
    nc.scalar.wait_ge(s_q, 1)
    nc.scalar.copy(q_sb[:, :], k_ps[:, :]).then_inc(s_qsb, 1)
    nc.scalar.wait_ge(s_s3, 1)
    nc.scalar.activation(
        w2[:, :], s3[:, :], ACTF.Sigmoid, bias=bias_sb[:, :], scale=SIG_SCALE
    ).then_inc(s_w2s, 1)
    nc.scalar.wait_ge(s_pbA, 1)
    nc.scalar.copy(vb[:, 0:M], pbA[:, :]).then_inc(s_vb0, 1)


    # ---- PE: warmups (garbage), q/k proj, v proj, broadcasts ----
    for _ in range(WARMUP_MMS):
        nc.tensor.matmul(wu_ps[:, :], wu[:, 0:P], wu[:, :], start=True, stop=True)

    def proj(p_t, x0, w0, src, wait, inc=None):
        last = None
        for c in range(NCH):
            if c == 0 and wait is not None:
                nc.tensor.wait_ge(wait, 16)
            last = nc.tensor.matmul(
                p_t[:, :],
                src[:, x0 + 2 * c : x0 + 2 * c + 2],
                src[:, w0 + c * M : w0 + (c + 1) * M],
                start=(c == 0),
                stop=(c == NCH - 1),
            )
        if inc is not None:
            last.then_inc(inc, 1)

    proj(k_ps, XK0, WQ0, pc_sb, s_pc1, inc=s_q)
    proj(q_ps, XQ0, WK0, pc_sb, s_pc2, inc=s_qk)
    # v: chunks 0..PA_SPLIT-1 from piece 1, rest from piece 2
    nc.tensor.wait_ge(s_pa1, 16)
    for c in range(NCH):
        if c == PA_SPLIT:
            nc.tensor.wait_ge(s_pa2, 16)
        mm = nc.tensor.matmul(
            v_ps[:, :],
            pa_sb[:, 2 * c : 2 * c + 2],
            pa_sb[:, 2 * NCH + c * M : 2 * NCH + (c + 1) * M],
            start=(c == 0),
            stop=(c == NCH - 1),
        )
    mm.then_inc(s_vps, 1)
    for c in range(NCH):
        mm = nc.tensor.matmul(
            v2_ps[:, :],
            pa_sb[:, XV2 + 2 * c : XV2 + 2 * c + 2],
            pa_sb[:, 2 * NCH + c * M : 2 * NCH + (c + 1) * M],
            start=(c == 0),
            stop=(c == NCH - 1),
        )
    mm.then_inc(s_v2, 1)
    nc.tensor.wait_ge(s_vrow, 1)
    nc.tensor.wait_ge(s_sel, 16)
    nc.tensor.matmul(
        pbA[:, :], sel[:, 0:P], vrow_sb[:, :], start=True, stop=True
    ).then_inc(s_pbA, 1)
    nc.tensor.matmul(
        pbB[:, :], sel[:, P : 2 * P], vrow_sb[:, :], start=True, stop=True
    ).then_inc(s_pbB, 1)
    # transpose row0 [2, 192] into partition-pair layout: partition p holds
    # elements {2p, 2p+1} of each batch (kv dhi-major column order = 2p+d)
    t1_ps = wu_ps[0:96, 0:2]
    t2_ps = wu_ps[0:96, 2:4]
    r0_pairs = row0_sb[0:2, 0:M].rearrange("p (m two) -> p two m", two=2)
    nc.tensor.wait_ge(s_row0, 1)
    nc.tensor.matmul(
        t1_ps, r0_pairs[:, 0:1, :], sel[:, 2 * P : 2 * P + 2],
        start=True, stop=True,
    ).then_inc(s_t1, 1)
    nc.tensor.matmul(
        t2_ps, r0_pairs[:, 1:2, :], sel[:, 2 * P : 2 * P + 2],
        start=True, stop=True,
    ).then_inc(s_t2, 1)

    # ---- Pool: preps (idx constants ride the pc load), triggers ----
    ctxi = pc_sb[:, CTXI0 : CTXI0 + 8].bitcast(i32)
    nc.gpsimd.wait_ge(s_pc1, 16)
    for b in range(2):
        nc.gpsimd.kv_writeback(
            out_d[b : b + 1, :, :].rearrange("b (p d) m -> b p d m", p=P),
            vb[:, b * M : (b + 1) * M]
            .rearrange("p (d b2 m) -> p d b2 m", d=1, b2=1)
            .broadcast_to([P, S // P, 1, M]),
            ctxi[:, 0:1],
            prepare_only=True,
            sem=kv_sem,
        ).then_inc(pq1, 1)
    nc.gpsimd.kv_writeback(
        row0_d[:, :].rearrange("b (p d n) -> b p d n", p=P, d=2),
        vbr[:, :].rearrange("p (d b n) -> p d b n", d=2, n=1),
        ctxi[:, 0:2],
        prepare_only=True,
        sem=r0_sem,
    ).then_inc(pq3, 1)
    nc.gpsimd.wait_ge(pq1, 2)
    nc.gpsimd.wait_ge(pq3, 1)
    nc.gpsimd.wait_ge(s_vb0, 1)
    nc.gpsimd.trigger_dma(count=1)
    nc.gpsimd.wait_ge(s_vb1, 1)
    nc.gpsimd.trigger_dma(count=1)
    nc.gpsimd.wait_ge(s_vbr, 1)
    nc.gpsimd.trigger_dma(count=1)
    nc.gpsimd.wait_ge(kv_sem, 32)
    nc.gpsimd.wait_ge(r0_sem, 16)

    nc.finalize()
    return nc


def _get_nc():
    with _lock:
        if "nc" not in _nc_cache:
            _nc_cache["nc"] = _build_nc()
        return _nc_cache["nc"]


def _prep_w(Wx, g, np_dt, scale=1.0):
    sl = np.asarray(Wx, dtype=np.float32)[HG * g : HG * (g + 1)].reshape(M, E)
    if scale != 1.0:
        sl = sl * scale
    return sl.T.reshape(NCH, P, M).transpose(1, 0, 2).reshape(P, NCH * M).astype(np_dt)


def _prep_x(x2, np_dt, scale=1.0):
    t = np.asarray(x2, dtype=np.float32)
    if scale != 1.0:
        t = t * scale
    return t.reshape(2, NCH, P).transpose(2, 1, 0).reshape(P, NCH * 2).astype(np_dt)


def kernel(query, key, value, Wq, Wk, Wv):
    global LAST_RESULTS
    from concourse.bass_utils import run_bass_kernel_spmd
    import concourse.mybir as mybir

    bf16 = np.dtype(mybir.dt.np(mybir.dt.bfloat16))
    fp8 = np.dtype(mybir.dt.np(mybir.dt.float8e4))

    query = np.asarray(query, dtype=np.float32).reshape(B, E)
    key = np.asarray(key, dtype=np.float32).reshape(B, E)
    value = np.asarray(value, dtype=np.float32).reshape(B, E)

    # constant idx words: ctxi = int32 zeros; sidx = int16 [0, 1, -1 x62]
    idx_bytes = np.zeros((P, 24), dtype=np.uint8)
    sidx = np.full((16, 8), -1, dtype=np.int16)
    sidx[0, 0] = 0
    sidx[1, 0] = 1
    idx_bytes[0:16, 8:24] = sidx.view(np.uint8)

    seld = np.zeros((2, 2 * P + 2), dtype=np.float32)
    seld[0, 0:P] = 1.0
    seld[1, P : 2 * P] = 1.0
    seld[0, 2 * P] = 1.0
    seld[1, 2 * P + 1] = 1.0
    seld = seld.astype(bf16)

    in_maps = []
    for c in range(N_CORES):
        g, bp = c % 4, c // 4
        pa = np.concatenate(
            [
                _prep_x(value[2 * bp : 2 * bp + 2], bf16, 1.0 / S),
                _prep_w(Wv, g, bf16),
                _prep_x(value[2 * bp : 2 * bp + 2], bf16, 1.0),
            ],
            axis=1,
        )
        pc8 = np.concatenate(
            [
                _prep_x(query[2 * bp : 2 * bp + 2], fp8, X8_SCALE),
                _prep_x(key[2 * bp : 2 * bp + 2], fp8, X8_SCALE),
                _prep_w(Wk, g, fp8, W8_SCALE),
                _prep_w(Wq, g, fp8, W8_SCALE),
            ],
            axis=1,
        )
        pcx = np.concatenate([idx_bytes, pc8.view(np.uint8)], axis=1).view(fp8)
        in_maps.append(
            {
                "pa": np.ascontiguousarray(pa),
                "pc": np.ascontiguousarray(pcx),
                "seld": seld,
            }
        )

    nc = _get_nc()
    LAST_RESULTS = run_bass_kernel_spmd(nc, in_maps, core_ids=list(range(N_CORES)))
    res = LAST_RESULTS.results

    full = np.empty((B, S, H * D), dtype=np.float32)
    for c in range(N_CORES):
        g, bp = c % 4, c // 4
        cols = slice(M * g, M * (g + 1))
        full[2 * bp : 2 * bp + 2, :, cols] = res[c]["out"].astype(np.float32)
        full[2 * bp : 2 * bp + 2, 0, cols] = res[c]["row0"][:, 0:M].astype(
            np.float32
        )
    return full
